# revision 15
# baseline (speedup 1.0000x reference)
"""DifferentialAttentionBlock on 8 NeuronCores — v2.

Sharding: DP on batch (cores 0-3 = batch 0, 4-7 = batch 1) x TP on heads
(4 heads per core).  Attention output shards are AllGathered within each
4-core batch group (not across all 8), in 6 half-chunk pieces so the
collectives overlap attention compute; each core then computes 256
output columns of its own batch via its Wo column shard.

Key structure (vs the v1 baseline, 246us -> ~150us):
  - all inputs host-pre-arranged to partition-major [128, X] layouts so
    every load DMA is a plain wide 2-D copy (no 256B-descriptor sprays);
    q/k loads interleaved on the sync ring, v + weights on scalar ring
  - PE warmup burst at t=0 (HAM un-throttles before the first real MM)
  - q/k projections share one PSUM pool, d-interleaved (PE never drains)
  - attention is software-pipelined: scores+exp for step s are emitted
    two h-steps ahead of the A@V of step s-2, keeping the PE queue full
    while ACT works through the exps (exp pairs two sk-tiles per instr)
  - the vvo stationary carries a 64-wide ones block, so each A@V matmul
    also produces the softmax colsum pre-broadcast across 64 partitions
    (rows 64:128 of the PSUM tile) at zero PE cost; normalization is
    then copy + reciprocal_approx_fast + mult + one scalar_tensor_tensor
    (folds the -lambda combine), all on DVE — gpsimd stays empty so the
    collective triggers never block anything
  - chunks of sq = [512, 256, 256]; each chunk ships its attnT in two
    half-AllGathers (2 heads each, triggered after h=1 and h=3); all
    Wo matmuls are emitted last and merge the gathered halves k-wise,
    so no engine FIFO ever waits on a collective mid-stream
"""

import math
import numpy as np

B, S, D = 2, 1024, 1024
H = 16
DH = 32          # q/k half head dim
DK = 64          # v head dim
HPC = 4          # heads per core
NCORES = 8
LAMBDA_INIT = 0.8 - 0.6 * math.exp(-0.3 * (1 - 1))
NSK = S // 128   # 8 s_k tiles
CHW = 512        # max sq chunk width
CHUNKS = [(0, 512), (512, 256), (768, 256)]
NCH = len(CHUNKS)
RG8 = [list(range(8))]
RG4 = [[0, 1, 2, 3], [4, 5, 6, 7]]

PROFILE = False
LAST_EXEC_NS = None
LAST_RESULTS = None

_cache = {}


def _try_install_ntff_hook():
    try:
        import sys, types
        import antenv
        try:
            import antenv.axon_hooks  # noqa: F401
            return
        except ImportError:
            pass
        mod = types.ModuleType("antenv.axon_hooks")
        mod._hook = None
        mod.set_axon_ntff_profile_hook = lambda h: setattr(mod, "_hook", h)
        mod.get_axon_ntff_profile_hook = lambda: mod._hook
        sys.modules["antenv.axon_hooks"] = mod
        antenv.axon_hooks = mod
        from trn_agent_boot.trn_boot import _ntff_profile_via_ctypes
        mod._hook = _ntff_profile_via_ctypes('/opt/axon/libaxon_pjrt.so')
    except Exception:
        pass


def _build(causal: bool):
    import concourse.bacc as bacc
    import concourse.mybir as mybir
    import concourse.tile as tile
    from concourse.tile_rust import add_dep_helper

    dt = mybir.dt
    f32, bf16 = dt.float32, dt.bfloat16
    AF = mybir.ActivationFunctionType
    OP = mybir.AluOpType

    nc = bacc.Bacc("TRN2", target_bir_lowering=False, debug=False,
                   num_devices=NCORES)

    def inp(name, shape, d=f32):
        return nc.dram_tensor(name, shape, d, kind="ExternalInput")

    # host-pre-arranged partition-major layouts
    qTh = inp("qTh", [128, 8 * S], bf16)
    kTh = inp("kTh", [128, 8 * S], bf16)
    vTh = inp("vTh", [128, 8 * S], bf16)
    Wq1 = inp("Wq1", [128, 8 * 128], bf16)
    Wq2 = inp("Wq2", [128, 8 * 128], bf16)
    Wk1 = inp("Wk1", [128, 8 * 128], bf16)
    Wk2 = inp("Wk2", [128, 8 * 128], bf16)
    Wv = inp("Wv", [128, 8 * 256], bf16)
    Wob = inp("Wob", [128, 8 * 256], bf16)      # my 256 output columns
    bq1 = inp("bq1", [128, 1]);  bq2 = inp("bq2", [128, 1])
    bk1 = inp("bk1", [128, 1]);  bk2 = inp("bk2", [128, 1])
    bv = inp("bv", [1, 256], bf16)
    ones_in = inp("ones1", [1, 128], bf16)
    triu = inp("triu", [128, 128], bf16)
    neglam = inp("neglam", [128, 1])
    maskT = None if causal else inp("maskT", [S, S])
    out_ext = nc.dram_tensor("out", [256, S], f32, kind="ExternalOutput")

    with tile.TileContext(nc) as tc:
        with (
            tc.tile_pool(name="const", bufs=1) as cpool,
            tc.tile_pool(name="wts", bufs=1) as wpool,
            tc.tile_pool(name="proj", bufs=1) as ppool,
            tc.tile_pool(name="acts", bufs=1) as apool,
            tc.tile_pool(name="edata", bufs=2) as epool,
            tc.tile_pool(name="small", bufs=2) as spool,
            tc.tile_pool(name="outs", bufs=2) as opool,
            tc.tile_pool(name="dram", bufs=1, space="DRAM") as dpool,
        ):
            # tiny AllGather issued first: absorbs cross-core launch skew
            dummy_in = dpool.tile([1, 16], bf16, name="dummy_in")
            dummy_out = dpool.tile([8, 16], bf16, name="dummy_out")
            nc.gpsimd.dma_start(dummy_in[:], triu[0:1, 0:16])
            nc.gpsimd.collective_compute(
                "AllGather", mybir.AluOpType.bypass, replica_groups=RG8,
                ins=[dummy_in.opt()], outs=[dummy_out.opt()])

            # ---- PE warmup: garbage matmuls while loads stream in,
            # so HAM un-throttles before the first real matmul ----
            wrm = wpool.tile([128, 640], bf16, tag="warm")
            nc.gpsimd.memset(wrm[:], 1.0)
            with tc.tile_pool(name="psWm", bufs=1, space="PSUM") as psWm:
                wps_ = psWm.tile([128, 256], f32, tag="w")
                for _ in range(20):
                    nc.tensor.matmul(wps_[:], wrm[:, 0:128],
                                     wrm[:, 384:640], start=True, stop=True)

            # ---- loads: q-path first so projections start ASAP ----
            wsb = {}
            for name, t in (("Wq1", Wq1), ("Wq2", Wq2),
                            ("Wk1", Wk1), ("Wk2", Wk2)):
                wsb[name] = wpool.tile([128, 8 * 128], bf16, tag=name,
                                       name=name)
                nc.scalar.dma_start(wsb[name][:], t[:, :])
            qsb = apool.tile([128, 8 * S], bf16, tag="qsb")
            ksb = apool.tile([128, 8 * S], bf16, tag="ksb")
            vsb = apool.tile([128, 8 * S], bf16, tag="vsb")
            for j in range(8):
                sl = slice(S * j, S * (j + 1))
                nc.sync.dma_start(qsb[:, sl], qTh[:, sl])
                nc.sync.dma_start(ksb[:, sl], kTh[:, sl])
            wv_sb = wpool.tile([128, 8 * 256], bf16, tag="Wv")
            nc.scalar.dma_start(wv_sb[:], Wv[:, :])
            for j in range(4):
                sl = slice(2 * S * j, 2 * S * (j + 1))
                nc.scalar.dma_start(vsb[:, sl], vTh[:, sl])
            wo_sb = wpool.tile([128, 8 * 256], bf16, tag="Wob")
            nc.scalar.dma_start(wo_sb[:], Wob[:, :])

            # constants on the gpsimd (SWDGE) queue
            triu_sb = cpool.tile([128, 128], bf16, tag="triu")
            nc.gpsimd.dma_start(triu_sb[:], triu[:, :])
            nlam_sb = cpool.tile([128, 1], f32, tag="neglam")
            nc.gpsimd.dma_start(nlam_sb[:], neglam[:, :])
            ones1 = cpool.tile([1, 128], bf16, tag="ones1")
            nc.gpsimd.dma_start(ones1[:], ones_in[:, :])
            bsb = {}
            for name, t in (("bq1", bq1), ("bq2", bq2), ("bk1", bk1),
                            ("bk2", bk2)):
                bsb[name] = cpool.tile([128, 1], f32, tag=name, name=name)
                nc.gpsimd.dma_start(bsb[name][:], t[:, :])
            bv_sb = cpool.tile([1, 256], bf16, tag="bv")
            nc.gpsimd.dma_start(bv_sb[:], bv[:, :])

            # kpad tiles zeroed early (DVE idle during load phase)
            kpad = {}
            for m_ in range(2):
                for h_ in range(4):
                    t_ = ppool.tile([128, S], bf16, tag=f"kp{m_}{h_}",
                                    name=f"kp{m_}{h_}")
                    nc.vector.memset(t_[:], 0.0)
                    kpad[(m_, h_)] = t_

            # ---- q + k projections (one pool, d-interleaved) ----
            q1T = ppool.tile([128, S], bf16, tag="q1T")
            q2T = ppool.tile([128, S], bf16, tag="q2T")
            with tc.tile_pool(name="psB", bufs=1, space="PSUM") as psB:
                pq1 = psB.tile([128, S], f32, tag="q1")
                pq2 = psB.tile([128, S], f32, tag="q2")
                pk1 = psB.tile([128, S], f32, tag="k1")
                pk2 = psB.tile([128, S], f32, tag="k2")
                for d in range(8):
                    qTd = qsb[:, d * S:(d + 1) * S]
                    kTd = ksb[:, d * S:(d + 1) * S]
                    for ps, wname, td in ((pq1, "Wq1", qTd),
                                          (pq2, "Wq2", qTd),
                                          (pk1, "Wk1", kTd),
                                          (pk2, "Wk2", kTd)):
                        lhsT = wsb[wname][:, d * 128:(d + 1) * 128]
                        for half in range(2):
                            nc.tensor.matmul(
                                ps[:, half * 512:(half + 1) * 512], lhsT,
                                td[:, half * 512:(half + 1) * 512],
                                start=(d == 0), stop=(d == 7))
                nc.scalar.activation(q1T[:], pq1[:], AF.Identity,
                                     bias=bsb["bq1"][:])
                nc.vector.tensor_scalar(q2T[:], pq2[:], bsb["bq2"][:],
                                        None, OP.add)
                for m, pk, bn in ((0, pk1, "bk1"), (1, pk2, "bk2")):
                    for h in range(4):
                        t = kpad[(m, h)]
                        sl = slice(32 * h, 32 * h + 32)
                        if m == 0:
                            nc.scalar.activation(t[sl, :], pk[sl, :],
                                                 AF.Identity,
                                                 bias=bsb[bn][sl, :])
                        else:
                            nc.vector.tensor_scalar(t[sl, :], pk[sl, :],
                                                    bsb[bn][sl, :],
                                                    None, OP.add)
            # ---- vv projection; vvo block i: 4 heads x (vv x64, 1) ----
            vvo = ppool.tile([128, 8 * 512], bf16, tag="vvo")
            with tc.tile_pool(name="psC", bufs=1, space="PSUM") as psC:
                pvv = [psC.tile([128, 256], f32, tag=f"vv{i}", name=f"vv{i}")
                       for i in range(8)]
                for d in range(8):
                    vTd = vsb[:, d * S:(d + 1) * S]
                    for i in range(8):
                        nc.tensor.matmul(
                            pvv[i][:], vTd[:, i * 128:(i + 1) * 128],
                            wv_sb[:, d * 256:(d + 1) * 256],
                            start=(d == 0), stop=False)
                for i in range(8):
                    nc.tensor.matmul(pvv[i][:], ones1[:], bv_sb[:],
                                     start=False, stop=True)
                    blk = vvo[:, i * 512:(i + 1) * 512]
                    blk3 = blk.rearrange("p (h c) -> p h c", c=128)
                    nc.vector.tensor_copy(
                        blk3[:, :, 0:64],
                        pvv[i].rearrange("p (h c) -> p h c", c=64))
                    nc.vector.memset(blk3[:, :, 64:128], 1.0)

            # ---- attention + per-chunk AllGather + Wo ----
            # Software-pipelined one h-step: scores+exp for step s are
            # emitted before the A@V of step s-1, so the PE always has a
            # dense run of matmuls while ACT works through the exps.
            qproj = (q1T, q2T)
            with (
                tc.tile_pool(name="psS", bufs=1, space="PSUM") as psS,
                tc.tile_pool(name="psO", bufs=2, space="PSUM") as psO,
                tc.tile_pool(name="mloc", bufs=2) as mpool,
            ):
                mT = {}
                aT = {}

                def chunk_meta(c):
                    cs, W = CHUNKS[c]
                    nvalid = min(NSK, (cs + W) // 128) if causal else NSK
                    ilist = list(range(nvalid))
                    return cs, W, [(ilist[x], ilist[x + 1])
                                   for x in range(0, nvalid, 2)]

                def emit_scores(c, h):
                    cs, W, pairs = chunk_meta(c)
                    if (not causal) and h == 0:
                        for i in range(NSK):
                            mT[(c, i)] = mpool.tile(
                                [128, CHW], f32, tag=f"mT{i%2}",
                                name=f"mT{c}{i}")
                            nc.sync.dma_start(
                                mT[(c, i)][:, 0:W],
                                maskT[i * 128:(i + 1) * 128, cs:cs + W])
                    ets = {}
                    for m in range(2):
                        etiles = []
                        for (i0, i1) in pairs:
                            ps = psS.tile([128, 2 * CHW], f32, tag="s",
                                          name=f"s{c}{m}{h}p{i0}", bufs=2)
                            for z, i in enumerate((i0, i1)):
                                lo = max(cs, 128 * i) if causal else cs
                                off = z * W + (lo - cs)
                                n = cs + W - lo
                                nc.tensor.matmul(
                                    ps[:, off:off + n],
                                    kpad[(m, h)][:, i * 128:(i + 1) * 128],
                                    qproj[m][:, lo:cs + W],
                                    start=True, stop=True)
                                if not causal:
                                    nc.vector.tensor_tensor(
                                        ps[:, off:off + n],
                                        ps[:, off:off + n],
                                        mT[(c, i)][:, lo - cs:W], OP.add)
                            e = epool.tile([128, 2 * CHW], bf16,
                                           tag=f"e{m}{(i0//2)%4}",
                                           name=f"e{m}h{h}p{i0}c{c}",
                                           bufs=3)
                            nc.scalar.activation(e[:, 0:2 * W],
                                                 ps[:, 0:2 * W], AF.Exp,
                                                 scale=0.125)
                            for z, i in enumerate((i0, i1)):
                                if causal and 128 * i >= cs:
                                    doff = z * W + 128 * i - cs
                                    nc.vector.tensor_tensor(
                                        e[:, doff:doff + 128],
                                        e[:, doff:doff + 128],
                                        triu_sb[:], OP.mult)
                            etiles.append(((i0, i1), e))
                        ets[m] = etiles
                    return ets

                def emit_av(c, h, ets):
                    cs, W, pairs = chunk_meta(c)
                    tfs = {}
                    for m in range(2):
                        # A@V in outT form; rows 64:128 get the colsum
                        # broadcast from the ones half of vvo
                        o = psO.tile([128, CHW], f32, tag=f"o{m}",
                                     name=f"o{m}h{h}c{c}")
                        x = 0
                        nmm = 2 * len(pairs)
                        for (i0, i1), e in ets[m]:
                            for z, i in enumerate((i0, i1)):
                                lo = max(cs, 128 * i) if causal else cs
                                nc.tensor.matmul(
                                    o[:, lo - cs:W],
                                    vvo[:, 512 * i + 128 * h:
                                        512 * i + 128 * h + 128],
                                    e[:, z * W + lo - cs:z * W + W],
                                    start=(x == 0), stop=(x == nmm - 1))
                                x += 1
                        cw = spool.tile([64, CHW], f32, tag=f"cw{m}",
                                        name=f"cw{m}h{h}c{c}")
                        nc.vector.tensor_copy(cw[:, 0:W], o[64:128, 0:W])
                        rb = spool.tile([64, CHW], f32, tag=f"rb{m}",
                                        name=f"rb{m}h{h}c{c}")
                        nc.vector.reciprocal_approx_fast(rb[:, 0:W],
                                                         cw[:, 0:W])
                        tf = spool.tile([64, CHW], f32, tag=f"tf{m}",
                                        name=f"tf{m}h{h}c{c}")
                        nc.vector.tensor_tensor(tf[:, 0:W], o[0:64, 0:W],
                                                rb[:, 0:W], OP.mult)
                        tfs[m] = tf
                    dst = aT[c][h // 2][64 * (h % 2):64 * (h % 2) + 64,
                                        0:W]
                    return nc.vector.scalar_tensor_tensor(
                        dst, tfs[1][:, 0:W], nlam_sb[0:64, 0:1],
                        tfs[0][:, 0:W], OP.mult, OP.add)

                agdone = {}

                def emit_ship(c, last_sub):
                    cs, W, _ = chunk_meta(c)
                    bounce = dpool.tile([256, W], bf16, name=f"bnc{c}")
                    for kk in range(2):
                        nc.sync.dma_start(
                            bounce[128 * kk:128 * (kk + 1), :],
                            aT[c][kk][:, 0:W])
                    ag = dpool.tile([4 * 256, W], bf16, name=f"ag{c}")
                    nc.gpsimd.collective_compute(
                        "AllGather", mybir.AluOpType.bypass,
                        replica_groups=RG4,
                        ins=[bounce.opt()], outs=[ag.opt()])
                    agdone[c] = (ag, last_sub)

                def emit_ship_half(c, kk, last_sub):
                    # ship one aT half (2 heads) as its own AllGather so
                    # the final chunk's gather starts before h=3 is done
                    cs, W, _ = chunk_meta(c)
                    bounce = dpool.tile([128, W], bf16, name=f"bnc{c}k{kk}")
                    nc.sync.dma_start(bounce[:], aT[c][kk][:, 0:W])
                    ag = dpool.tile([4 * 128, W], bf16, name=f"ag{c}k{kk}")
                    nc.gpsimd.collective_compute(
                        "AllGather", mybir.AluOpType.bypass,
                        replica_groups=RG4,
                        ins=[bounce.opt()], outs=[ag.opt()])
                    agdone[(c, kk)] = (ag, last_sub)

                def emit_wo(c):
                    cs, W, _ = chunk_meta(c)
                    korder = []
                    for kk in range(2):
                        ag, last_sub = agdone[(c, kk)]
                        myt = mpool.tile([128, 4 * CHW], bf16,
                                         tag=f"myth{kk}",
                                         name=f"myt{c}k{kk}", bufs=2)
                        myv = myt[:, 0:4 * W]
                        mydma = nc.scalar.dma_start(
                            myv.rearrange("p (k x) -> p k x", k=4),
                            ag.rearrange("(k p) x -> p k x", p=128))
                        add_dep_helper(mydma.ins, last_sub.ins,
                                       reason="wo after normalize")
                        for r in range(4):
                            korder.append((2 * r + kk,
                                           myv[:, W * r:W * (r + 1)]))
                    for cg in range(2):
                        wps = psO.tile([128, CHW], f32, tag=f"o{cg}",
                                       name=f"wo{c}{cg}")
                        for x, (k, rhs) in enumerate(korder):
                            nc.tensor.matmul(
                                wps[:, 0:W],
                                wo_sb[:, 256 * k + 128 * cg:
                                      256 * k + 128 * (cg + 1)],
                                rhs, start=(x == 0), stop=(x == 7))
                        osb = opool.tile([128, CHW], f32, tag="osb",
                                         name=f"osb{c}{cg}")
                        if cg == 0:
                            nc.scalar.copy(osb[:, 0:W], wps[:, 0:W])
                        else:
                            nc.vector.tensor_copy(osb[:, 0:W],
                                                  wps[:, 0:W])
                        nc.scalar.dma_start(
                            out_ext[128 * cg:128 * (cg + 1),
                                    cs:cs + W], osb[:, 0:W])

                steps = [(c, h) for c in range(NCH) for h in range(4)]
                pending = []

                def do_av(pc, ph, pets):
                    sub = emit_av(pc, ph, pets)
                    if ph == 3:
                        emit_ship_half(pc, 1, sub)
                    elif ph == 1:
                        emit_ship_half(pc, 0, sub)

                for (c, h) in steps:
                    if h == 0:
                        aT[c] = [opool.tile([128, CHW], bf16,
                                            tag=f"aT{kk}",
                                            name=f"aT{kk}c{c}")
                                 for kk in range(2)]
                    ets = emit_scores(c, h)
                    pending.append((c, h, ets))
                    if len(pending) > 2:
                        do_av(*pending.pop(0))
                while pending:
                    do_av(*pending.pop(0))
                for c in range(NCH):
                    emit_wo(c)

    nc.compile()
    return nc


def kernel(**inputs):
    global LAST_EXEC_NS
    import ml_dtypes

    q = np.asarray(inputs["q"], dtype=np.float32)
    k = np.asarray(inputs["k"], dtype=np.float32)
    v = np.asarray(inputs["v"], dtype=np.float32)
    mask = np.asarray(inputs["mask"])
    f32 = np.float32
    Wq1f = np.asarray(inputs["Wq1"], f32); Wq2f = np.asarray(inputs["Wq2"], f32)
    Wk1f = np.asarray(inputs["Wk1"], f32); Wk2f = np.asarray(inputs["Wk2"], f32)
    Wvf = np.asarray(inputs["Wv"], f32);   Wof = np.asarray(inputs["Wo"], f32)
    bq1f = np.asarray(inputs["bq1"], f32); bq2f = np.asarray(inputs["bq2"], f32)
    bk1f = np.asarray(inputs["bk1"], f32); bk2f = np.asarray(inputs["bk2"], f32)
    bvf = np.asarray(inputs["bv"], f32);   bof = np.asarray(inputs["bo"], f32)
    lam = float(np.exp(float(inputs["lq1"][0]) * float(inputs["lk1"][0]))
                - np.exp(float(inputs["lq2"][0]) * float(inputs["lk2"][0]))
                + LAMBDA_INIT)

    mk = (mask.reshape(B, S, S) != 0)
    causal = bool((mk == np.tril(np.ones((S, S), bool))[None]).all())

    key = "causal" if causal else "general"
    if key not in _cache:
        _cache[key] = _build(causal)
    nc = _cache[key]

    bfl = ml_dtypes.bfloat16

    def pmaj(x, width):
        # [1024, width] -> [128, 8*width] partition-major over 8 d-tiles
        return np.ascontiguousarray(
            x.reshape(8, 128, width).transpose(1, 0, 2).reshape(
                128, 8 * width)).astype(bfl)

    qTl = [pmaj(q[b].T, S) for b in range(B)]
    kTl = [pmaj(k[b].T, S) for b in range(B)]
    vTl = [pmaj(v[b].T, S) for b in range(B)]
    triu = np.triu(np.ones((128, 128))).astype(bfl)
    nlam = np.full((128, 1), -lam, f32)
    maskTs = None
    if not causal:
        maskTs = [np.ascontiguousarray(
            np.where(mk[b], np.float32(0), np.float32(-1e9)).T)
            for b in range(B)]

    in_maps = []
    for c in range(NCORES):
        b, g = divmod(c, 4)
        im = dict(
            qTh=qTl[b], kTh=kTl[b], vTh=vTl[b],
            Wq1=pmaj(Wq1f[:, 128 * g:128 * (g + 1)], 128),
            Wq2=pmaj(Wq2f[:, 128 * g:128 * (g + 1)], 128),
            Wk1=pmaj(Wk1f[:, 128 * g:128 * (g + 1)], 128),
            Wk2=pmaj(Wk2f[:, 128 * g:128 * (g + 1)], 128),
            Wv=pmaj(Wvf[:, 256 * g:256 * (g + 1)], 256),
            Wob=pmaj(Wof[:, 256 * g:256 * (g + 1)], 256),
            bq1=np.ascontiguousarray(bq1f[128 * g:128 * (g + 1)]).reshape(128, 1),
            bq2=np.ascontiguousarray(bq2f[128 * g:128 * (g + 1)]).reshape(128, 1),
            bk1=np.ascontiguousarray(bk1f[128 * g:128 * (g + 1)]).reshape(128, 1),
            bk2=np.ascontiguousarray(bk2f[128 * g:128 * (g + 1)]).reshape(128, 1),
            bv=np.ascontiguousarray(bvf[256 * g:256 * (g + 1)]).reshape(1, 256).astype(bfl),
            triu=triu, neglam=nlam,
            ones1=np.ones((1, 128), bfl),
        )
        if not causal:
            im["maskT"] = maskTs[b]
        in_maps.append(im)

    from concourse.bass_utils import run_bass_kernel_spmd
    if PROFILE:
        _try_install_ntff_hook()
        res = run_bass_kernel_spmd(nc, in_maps, list(range(NCORES)),
                                   trace=True)
        LAST_EXEC_NS = res.exec_time_ns
        globals()["LAST_RESULTS"] = res
    else:
        res = run_bass_kernel_spmd(nc, in_maps, list(range(NCORES)))

    out = np.empty((B, S, D), np.float32)
    for c in range(NCORES):
        b, g = divmod(c, 4)
        o = res.results[c]["out"]          # [256 cols, 1024 pos]
        out[b, :, 256 * g:256 * (g + 1)] = o.T
    out += bof[None, None, :]
    return out


# revision 16
# speedup vs baseline: 1.2039x; 1.2039x over previous
"""DifferentialAttentionBlock on 8 NeuronCores — v2.

Sharding: DP on batch (cores 0-3 = batch 0, 4-7 = batch 1) x TP on heads
(4 heads per core).  Attention output shards are AllGathered within each
4-core batch group (not across all 8), in 6 half-chunk pieces so the
collectives overlap attention compute; each core then computes 256
output columns of its own batch via its Wo column shard.

Key structure (vs the v1 baseline, 246us -> ~150us):
  - all inputs host-pre-arranged to partition-major [128, X] layouts so
    every load DMA is a plain wide 2-D copy (no 256B-descriptor sprays);
    q/k loads interleaved on the sync ring, v + weights on scalar ring
  - PE warmup burst at t=0 (HAM un-throttles before the first real MM)
  - q/k projections share one PSUM pool, d-interleaved (PE never drains)
  - attention is software-pipelined: scores+exp for step s are emitted
    two h-steps ahead of the A@V of step s-2, keeping the PE queue full
    while ACT works through the exps (exp pairs two sk-tiles per instr)
  - the vvo stationary carries a 64-wide ones block, so each A@V matmul
    also produces the softmax colsum pre-broadcast across 64 partitions
    (rows 64:128 of the PSUM tile) at zero PE cost; normalization is
    then copy + reciprocal_approx_fast + mult + one scalar_tensor_tensor
    (folds the -lambda combine), all on DVE — gpsimd stays empty so the
    collective triggers never block anything
  - chunks of sq = [512, 256, 256]; each chunk ships its attnT in two
    half-AllGathers (2 heads each, triggered after h=1 and h=3); all
    Wo matmuls are emitted last and merge the gathered halves k-wise,
    so no engine FIFO ever waits on a collective mid-stream
"""

import math
import numpy as np

B, S, D = 2, 1024, 1024
H = 16
DH = 32          # q/k half head dim
DK = 64          # v head dim
HPC = 4          # heads per core
NCORES = 8
LAMBDA_INIT = 0.8 - 0.6 * math.exp(-0.3 * (1 - 1))
NSK = S // 128   # 8 s_k tiles
CHW = 512        # max sq chunk width
CHUNKS = [(0, 512), (512, 256), (768, 256)]
NCH = len(CHUNKS)
RG8 = [list(range(8))]
RG4 = [[0, 1, 2, 3], [4, 5, 6, 7]]

PROFILE = False
LAST_EXEC_NS = None
LAST_RESULTS = None

_cache = {}


def _try_install_ntff_hook():
    try:
        import sys, types
        import antenv
        try:
            import antenv.axon_hooks  # noqa: F401
            return
        except ImportError:
            pass
        mod = types.ModuleType("antenv.axon_hooks")
        mod._hook = None
        mod.set_axon_ntff_profile_hook = lambda h: setattr(mod, "_hook", h)
        mod.get_axon_ntff_profile_hook = lambda: mod._hook
        sys.modules["antenv.axon_hooks"] = mod
        antenv.axon_hooks = mod
        from trn_agent_boot.trn_boot import _ntff_profile_via_ctypes
        mod._hook = _ntff_profile_via_ctypes('/opt/axon/libaxon_pjrt.so')
    except Exception:
        pass


def _build(causal: bool):
    import concourse.bacc as bacc
    import concourse.mybir as mybir
    import concourse.tile as tile
    from concourse.tile_rust import add_dep_helper

    dt = mybir.dt
    f32, bf16 = dt.float32, dt.bfloat16
    AF = mybir.ActivationFunctionType
    OP = mybir.AluOpType

    nc = bacc.Bacc("TRN2", target_bir_lowering=False, debug=False,
                   num_devices=NCORES)

    def inp(name, shape, d=f32):
        return nc.dram_tensor(name, shape, d, kind="ExternalInput")

    # host-pre-arranged partition-major layouts
    qTh = inp("qTh", [128, 8 * S], bf16)
    kTh = inp("kTh", [128, 8 * S], bf16)
    vTh = inp("vTh", [128, 8 * S], bf16)
    Wq1 = inp("Wq1", [128, 8 * 128], bf16)
    Wq2 = inp("Wq2", [128, 8 * 128], bf16)
    Wk1 = inp("Wk1", [128, 8 * 128], bf16)
    Wk2 = inp("Wk2", [128, 8 * 128], bf16)
    Wv = inp("Wv", [128, 8 * 256], bf16)
    Wob = inp("Wob", [128, 8 * 256], bf16)      # my 256 output columns
    bq1 = inp("bq1", [128, 1]);  bq2 = inp("bq2", [128, 1])
    bk1 = inp("bk1", [128, 1]);  bk2 = inp("bk2", [128, 1])
    bv = inp("bv", [1, 256], bf16)
    ones_in = inp("ones1", [1, 128], bf16)
    triu = inp("triu", [128, 128], bf16)
    neglam = inp("neglam", [128, 1])
    maskT = None if causal else inp("maskT", [S, S])
    out_ext = nc.dram_tensor("out", [256, S], f32, kind="ExternalOutput")

    with tile.TileContext(nc) as tc:
        with (
            tc.tile_pool(name="const", bufs=1) as cpool,
            tc.tile_pool(name="wts", bufs=1) as wpool,
            tc.tile_pool(name="proj", bufs=1) as ppool,
            tc.tile_pool(name="acts", bufs=1) as apool,
            tc.tile_pool(name="edata", bufs=2) as epool,
            tc.tile_pool(name="small", bufs=2) as spool,
            tc.tile_pool(name="outs", bufs=2) as opool,
            tc.tile_pool(name="dram", bufs=1, space="DRAM") as dpool,
        ):
            # tiny AllGather issued first: absorbs cross-core launch skew
            dummy_in = dpool.tile([1, 16], bf16, name="dummy_in")
            dummy_out = dpool.tile([8, 16], bf16, name="dummy_out")
            nc.gpsimd.dma_start(dummy_in[:], triu[0:1, 0:16])
            nc.gpsimd.collective_compute(
                "AllGather", mybir.AluOpType.bypass, replica_groups=RG8,
                ins=[dummy_in.opt()], outs=[dummy_out.opt()])

            # ---- PE warmup: garbage matmuls while loads stream in,
            # so HAM un-throttles before the first real matmul ----
            wrm = wpool.tile([128, 640], bf16, tag="warm")
            nc.gpsimd.memset(wrm[:], 1.0)
            with tc.tile_pool(name="psWm", bufs=1, space="PSUM") as psWm:
                wps_ = psWm.tile([128, 256], f32, tag="w")
                for _ in range(20):
                    nc.tensor.matmul(wps_[:], wrm[:, 0:128],
                                     wrm[:, 384:640], start=True, stop=True)

            # ---- loads: q-path first so projections start ASAP ----
            wsb = {}
            for name, t in (("Wq1", Wq1), ("Wq2", Wq2),
                            ("Wk1", Wk1), ("Wk2", Wk2)):
                wsb[name] = wpool.tile([128, 8 * 128], bf16, tag=name,
                                       name=name)
                nc.scalar.dma_start(wsb[name][:], t[:, :])
            qsb = apool.tile([128, 8 * S], bf16, tag="qsb")
            ksb = apool.tile([128, 8 * S], bf16, tag="ksb")
            vsb = apool.tile([128, 8 * S], bf16, tag="vsb")
            for j in range(8):
                sl = slice(S * j, S * (j + 1))
                nc.sync.dma_start(qsb[:, sl], qTh[:, sl])
                nc.sync.dma_start(ksb[:, sl], kTh[:, sl])
            wv_sb = wpool.tile([128, 8 * 256], bf16, tag="Wv")
            nc.scalar.dma_start(wv_sb[:], Wv[:, :])
            for j in range(4):
                sl = slice(2 * S * j, 2 * S * (j + 1))
                nc.scalar.dma_start(vsb[:, sl], vTh[:, sl])
            wo_sb = wpool.tile([128, 8 * 256], bf16, tag="Wob")
            nc.scalar.dma_start(wo_sb[:], Wob[:, :])

            # constants on the gpsimd (SWDGE) queue
            triu_sb = cpool.tile([128, 128], bf16, tag="triu")
            nc.gpsimd.dma_start(triu_sb[:], triu[:, :])
            nlam_sb = cpool.tile([128, 1], f32, tag="neglam")
            nc.gpsimd.dma_start(nlam_sb[:], neglam[:, :])
            ones1 = cpool.tile([1, 128], bf16, tag="ones1")
            nc.gpsimd.dma_start(ones1[:], ones_in[:, :])
            bsb = {}
            for name, t in (("bq1", bq1), ("bq2", bq2), ("bk1", bk1),
                            ("bk2", bk2)):
                bsb[name] = cpool.tile([128, 1], f32, tag=name, name=name)
                nc.gpsimd.dma_start(bsb[name][:], t[:, :])
            bv_sb = cpool.tile([1, 256], bf16, tag="bv")
            nc.gpsimd.dma_start(bv_sb[:], bv[:, :])

            # kpad tiles zeroed early (DVE idle during load phase)
            kpad = {}
            for m_ in range(2):
                for h_ in range(4):
                    t_ = ppool.tile([128, S], bf16, tag=f"kp{m_}{h_}",
                                    name=f"kp{m_}{h_}")
                    nc.vector.memset(t_[:], 0.0)
                    kpad[(m_, h_)] = t_

            # ---- q + k projections (one pool, d-interleaved) ----
            q1T = ppool.tile([128, S], bf16, tag="q1T")
            q2T = ppool.tile([128, S], bf16, tag="q2T")
            with tc.tile_pool(name="psB", bufs=1, space="PSUM") as psB:
                pq1 = psB.tile([128, S], f32, tag="q1")
                pq2 = psB.tile([128, S], f32, tag="q2")
                pk1 = psB.tile([128, S], f32, tag="k1")
                pk2 = psB.tile([128, S], f32, tag="k2")
                for d in range(8):
                    qTd = qsb[:, d * S:(d + 1) * S]
                    kTd = ksb[:, d * S:(d + 1) * S]
                    for ps, wname, td in ((pq1, "Wq1", qTd),
                                          (pq2, "Wq2", qTd),
                                          (pk1, "Wk1", kTd),
                                          (pk2, "Wk2", kTd)):
                        lhsT = wsb[wname][:, d * 128:(d + 1) * 128]
                        for half in range(2):
                            nc.tensor.matmul(
                                ps[:, half * 512:(half + 1) * 512], lhsT,
                                td[:, half * 512:(half + 1) * 512],
                                start=(d == 0), stop=(d == 7))
                nc.scalar.activation(q1T[:], pq1[:], AF.Identity,
                                     bias=bsb["bq1"][:])
                nc.vector.tensor_scalar(q2T[:], pq2[:], bsb["bq2"][:],
                                        None, OP.add)
                for m, pk, bn in ((0, pk1, "bk1"), (1, pk2, "bk2")):
                    for h in range(4):
                        t = kpad[(m, h)]
                        sl = slice(32 * h, 32 * h + 32)
                        if m == 0:
                            nc.scalar.activation(t[sl, :], pk[sl, :],
                                                 AF.Identity,
                                                 bias=bsb[bn][sl, :])
                        else:
                            nc.vector.tensor_scalar(t[sl, :], pk[sl, :],
                                                    bsb[bn][sl, :],
                                                    None, OP.add)
            # ---- vv projection; vvo block i: 4 heads x (vv x64, 1) ----
            vvo = ppool.tile([128, 8 * 512], bf16, tag="vvo")
            with tc.tile_pool(name="psC", bufs=1, space="PSUM") as psC:
                pvv = [psC.tile([128, 256], f32, tag=f"vv{i}", name=f"vv{i}")
                       for i in range(8)]
                for d in range(8):
                    vTd = vsb[:, d * S:(d + 1) * S]
                    for i in range(8):
                        nc.tensor.matmul(
                            pvv[i][:], vTd[:, i * 128:(i + 1) * 128],
                            wv_sb[:, d * 256:(d + 1) * 256],
                            start=(d == 0), stop=False)
                for i in range(8):
                    nc.tensor.matmul(pvv[i][:], ones1[:], bv_sb[:],
                                     start=False, stop=True)
                    blk = vvo[:, i * 512:(i + 1) * 512]
                    blk3 = blk.rearrange("p (h c) -> p h c", c=128)
                    nc.vector.tensor_copy(
                        blk3[:, :, 0:64],
                        pvv[i].rearrange("p (h c) -> p h c", c=64))
                    nc.vector.memset(blk3[:, :, 64:128], 1.0)

            # ---- attention + per-chunk AllGather + Wo ----
            # Software-pipelined one h-step: scores+exp for step s are
            # emitted before the A@V of step s-1, so the PE always has a
            # dense run of matmuls while ACT works through the exps.
            qproj = (q1T, q2T)
            with (
                tc.tile_pool(name="psS", bufs=1, space="PSUM") as psS,
                tc.tile_pool(name="psO", bufs=2, space="PSUM") as psO,
                tc.tile_pool(name="mloc", bufs=2) as mpool,
            ):
                mT = {}
                aT = {}

                def chunk_meta(c):
                    cs, W = CHUNKS[c]
                    nvalid = min(NSK, (cs + W) // 128) if causal else NSK
                    ilist = list(range(nvalid))
                    return cs, W, [(ilist[x], ilist[x + 1])
                                   for x in range(0, nvalid, 2)]

                def emit_scores(c, h):
                    cs, W, pairs = chunk_meta(c)
                    if (not causal) and h == 0:
                        for i in range(NSK):
                            mT[(c, i)] = mpool.tile(
                                [128, CHW], f32, tag=f"mT{i%2}",
                                name=f"mT{c}{i}")
                            nc.sync.dma_start(
                                mT[(c, i)][:, 0:W],
                                maskT[i * 128:(i + 1) * 128, cs:cs + W])
                    ets = {}
                    for m in range(2):
                        etiles = []
                        for (i0, i1) in pairs:
                            ps = psS.tile([128, 2 * CHW], f32, tag="s",
                                          name=f"s{c}{m}{h}p{i0}", bufs=2)
                            for z, i in enumerate((i0, i1)):
                                lo = max(cs, 128 * i) if causal else cs
                                off = z * W + (lo - cs)
                                n = cs + W - lo
                                nc.tensor.matmul(
                                    ps[:, off:off + n],
                                    kpad[(m, h)][:, i * 128:(i + 1) * 128],
                                    qproj[m][:, lo:cs + W],
                                    start=True, stop=True)
                                if not causal:
                                    nc.vector.tensor_tensor(
                                        ps[:, off:off + n],
                                        ps[:, off:off + n],
                                        mT[(c, i)][:, lo - cs:W], OP.add)
                            e = epool.tile([128, 2 * CHW], bf16,
                                           tag=f"e{m}{(i0//2)%4}",
                                           name=f"e{m}h{h}p{i0}c{c}",
                                           bufs=3)
                            nc.scalar.activation(e[:, 0:2 * W],
                                                 ps[:, 0:2 * W], AF.Exp,
                                                 scale=0.125)
                            for z, i in enumerate((i0, i1)):
                                if causal and 128 * i >= cs:
                                    doff = z * W + 128 * i - cs
                                    nc.vector.tensor_tensor(
                                        e[:, doff:doff + 128],
                                        e[:, doff:doff + 128],
                                        triu_sb[:], OP.mult)
                            etiles.append(((i0, i1), e))
                        ets[m] = etiles
                    return ets

                def emit_av(c, h, ets):
                    cs, W, pairs = chunk_meta(c)
                    tfs = {}
                    for m in range(2):
                        # A@V in outT form; rows 64:128 get the colsum
                        # broadcast from the ones half of vvo
                        o = psO.tile([128, CHW], f32, tag=f"o{m}",
                                     name=f"o{m}h{h}c{c}")
                        x = 0
                        nmm = 2 * len(pairs)
                        for (i0, i1), e in ets[m]:
                            for z, i in enumerate((i0, i1)):
                                lo = max(cs, 128 * i) if causal else cs
                                nc.tensor.matmul(
                                    o[:, lo - cs:W],
                                    vvo[:, 512 * i + 128 * h:
                                        512 * i + 128 * h + 128],
                                    e[:, z * W + lo - cs:z * W + W],
                                    start=(x == 0), stop=(x == nmm - 1))
                                x += 1
                        cw = spool.tile([64, CHW], f32, tag=f"cw{m}",
                                        name=f"cw{m}h{h}c{c}")
                        nc.vector.tensor_copy(cw[:, 0:W], o[64:128, 0:W])
                        rb = spool.tile([64, CHW], f32, tag=f"rb{m}",
                                        name=f"rb{m}h{h}c{c}")
                        nc.vector.reciprocal_approx_fast(rb[:, 0:W],
                                                         cw[:, 0:W])
                        tf = spool.tile([64, CHW], f32, tag=f"tf{m}",
                                        name=f"tf{m}h{h}c{c}")
                        nc.vector.tensor_tensor(tf[:, 0:W], o[0:64, 0:W],
                                                rb[:, 0:W], OP.mult)
                        tfs[m] = tf
                    dst = aT[c][h // 2][64 * (h % 2):64 * (h % 2) + 64,
                                        0:W]
                    return nc.vector.scalar_tensor_tensor(
                        dst, tfs[1][:, 0:W], nlam_sb[0:64, 0:1],
                        tfs[0][:, 0:W], OP.mult, OP.add)

                agdone = {}

                def emit_ship(c, last_sub):
                    cs, W, _ = chunk_meta(c)
                    bounce = dpool.tile([256, W], bf16, name=f"bnc{c}")
                    for kk in range(2):
                        nc.sync.dma_start(
                            bounce[128 * kk:128 * (kk + 1), :],
                            aT[c][kk][:, 0:W])
                    ag = dpool.tile([4 * 256, W], bf16, name=f"ag{c}")
                    nc.gpsimd.collective_compute(
                        "AllGather", mybir.AluOpType.bypass,
                        replica_groups=RG4,
                        ins=[bounce.opt()], outs=[ag.opt()])
                    agdone[c] = (ag, last_sub)

                def emit_ship_half(c, kk, last_sub):
                    # ship one aT half (2 heads) as its own AllGather so
                    # the final chunk's gather starts before h=3 is done
                    cs, W, _ = chunk_meta(c)
                    bounce = dpool.tile([128, W], bf16, name=f"bnc{c}k{kk}")
                    nc.sync.dma_start(bounce[:], aT[c][kk][:, 0:W])
                    ag = dpool.tile([4 * 128, W], bf16, name=f"ag{c}k{kk}")
                    nc.gpsimd.collective_compute(
                        "AllGather", mybir.AluOpType.bypass,
                        replica_groups=RG4,
                        ins=[bounce.opt()], outs=[ag.opt()])
                    agdone[(c, kk)] = (ag, last_sub)

                mytd = {}

                def emit_myt(c):
                    cs, W, _ = chunk_meta(c)
                    for kk in range(2):
                        ag, last_sub = agdone[(c, kk)]
                        myt = mpool.tile([128, 4 * CHW], bf16,
                                         tag=f"myth{kk}",
                                         name=f"myt{c}k{kk}", bufs=3)
                        myv = myt[:, 0:4 * W]
                        mydma = nc.sync.dma_start(
                            myv.rearrange("p (k x) -> p k x", k=4),
                            ag.rearrange("(k p) x -> p k x", p=128))
                        add_dep_helper(mydma.ins, last_sub.ins,
                                       reason="wo after normalize")
                        mytd[(c, kk)] = myv

                def emit_wo(c):
                    cs, W, _ = chunk_meta(c)
                    korder = []
                    for kk in range(2):
                        myv = mytd[(c, kk)]
                        for r in range(4):
                            korder.append((2 * r + kk,
                                           myv[:, W * r:W * (r + 1)]))
                    for cg in range(2):
                        wps = psO.tile([128, CHW], f32, tag=f"o{cg}",
                                       name=f"wo{c}{cg}")
                        for x, (k, rhs) in enumerate(korder):
                            nc.tensor.matmul(
                                wps[:, 0:W],
                                wo_sb[:, 256 * k + 128 * cg:
                                      256 * k + 128 * (cg + 1)],
                                rhs, start=(x == 0), stop=(x == 7))
                        osb = opool.tile([128, CHW], f32, tag="osb",
                                         name=f"osb{c}{cg}")
                        if cg == 0:
                            nc.scalar.copy(osb[:, 0:W], wps[:, 0:W])
                        else:
                            nc.vector.tensor_copy(osb[:, 0:W],
                                                  wps[:, 0:W])
                        nc.scalar.dma_start(
                            out_ext[128 * cg:128 * (cg + 1),
                                    cs:cs + W], osb[:, 0:W])

                steps = [(c, h) for c in range(NCH) for h in range(4)]
                pending = []

                def do_av(pc, ph, pets):
                    sub = emit_av(pc, ph, pets)
                    if ph == 3:
                        emit_ship_half(pc, 1, sub)
                        if pc >= 1:
                            emit_myt(pc - 1)
                    elif ph == 1:
                        emit_ship_half(pc, 0, sub)

                for (c, h) in steps:
                    if h == 0:
                        aT[c] = [opool.tile([128, CHW], bf16,
                                            tag=f"aT{kk}",
                                            name=f"aT{kk}c{c}")
                                 for kk in range(2)]
                    ets = emit_scores(c, h)
                    pending.append((c, h, ets))
                    if len(pending) > 2:
                        do_av(*pending.pop(0))
                while pending:
                    do_av(*pending.pop(0))
                emit_myt(NCH - 1)
                for c in range(NCH):
                    emit_wo(c)

    nc.compile()
    return nc


def kernel(**inputs):
    global LAST_EXEC_NS
    import ml_dtypes

    q = np.asarray(inputs["q"], dtype=np.float32)
    k = np.asarray(inputs["k"], dtype=np.float32)
    v = np.asarray(inputs["v"], dtype=np.float32)
    mask = np.asarray(inputs["mask"])
    f32 = np.float32
    Wq1f = np.asarray(inputs["Wq1"], f32); Wq2f = np.asarray(inputs["Wq2"], f32)
    Wk1f = np.asarray(inputs["Wk1"], f32); Wk2f = np.asarray(inputs["Wk2"], f32)
    Wvf = np.asarray(inputs["Wv"], f32);   Wof = np.asarray(inputs["Wo"], f32)
    bq1f = np.asarray(inputs["bq1"], f32); bq2f = np.asarray(inputs["bq2"], f32)
    bk1f = np.asarray(inputs["bk1"], f32); bk2f = np.asarray(inputs["bk2"], f32)
    bvf = np.asarray(inputs["bv"], f32);   bof = np.asarray(inputs["bo"], f32)
    lam = float(np.exp(float(inputs["lq1"][0]) * float(inputs["lk1"][0]))
                - np.exp(float(inputs["lq2"][0]) * float(inputs["lk2"][0]))
                + LAMBDA_INIT)

    mk = (mask.reshape(B, S, S) != 0)
    causal = bool((mk == np.tril(np.ones((S, S), bool))[None]).all())

    key = "causal" if causal else "general"
    if key not in _cache:
        _cache[key] = _build(causal)
    nc = _cache[key]

    bfl = ml_dtypes.bfloat16

    def pmaj(x, width):
        # [1024, width] -> [128, 8*width] partition-major over 8 d-tiles
        return np.ascontiguousarray(
            x.reshape(8, 128, width).transpose(1, 0, 2).reshape(
                128, 8 * width)).astype(bfl)

    qTl = [pmaj(q[b].T, S) for b in range(B)]
    kTl = [pmaj(k[b].T, S) for b in range(B)]
    vTl = [pmaj(v[b].T, S) for b in range(B)]
    triu = np.triu(np.ones((128, 128))).astype(bfl)
    nlam = np.full((128, 1), -lam, f32)
    maskTs = None
    if not causal:
        maskTs = [np.ascontiguousarray(
            np.where(mk[b], np.float32(0), np.float32(-1e9)).T)
            for b in range(B)]

    in_maps = []
    for c in range(NCORES):
        b, g = divmod(c, 4)
        im = dict(
            qTh=qTl[b], kTh=kTl[b], vTh=vTl[b],
            Wq1=pmaj(Wq1f[:, 128 * g:128 * (g + 1)], 128),
            Wq2=pmaj(Wq2f[:, 128 * g:128 * (g + 1)], 128),
            Wk1=pmaj(Wk1f[:, 128 * g:128 * (g + 1)], 128),
            Wk2=pmaj(Wk2f[:, 128 * g:128 * (g + 1)], 128),
            Wv=pmaj(Wvf[:, 256 * g:256 * (g + 1)], 256),
            Wob=pmaj(Wof[:, 256 * g:256 * (g + 1)], 256),
            bq1=np.ascontiguousarray(bq1f[128 * g:128 * (g + 1)]).reshape(128, 1),
            bq2=np.ascontiguousarray(bq2f[128 * g:128 * (g + 1)]).reshape(128, 1),
            bk1=np.ascontiguousarray(bk1f[128 * g:128 * (g + 1)]).reshape(128, 1),
            bk2=np.ascontiguousarray(bk2f[128 * g:128 * (g + 1)]).reshape(128, 1),
            bv=np.ascontiguousarray(bvf[256 * g:256 * (g + 1)]).reshape(1, 256).astype(bfl),
            triu=triu, neglam=nlam,
            ones1=np.ones((1, 128), bfl),
        )
        if not causal:
            im["maskT"] = maskTs[b]
        in_maps.append(im)

    from concourse.bass_utils import run_bass_kernel_spmd
    if PROFILE:
        _try_install_ntff_hook()
        res = run_bass_kernel_spmd(nc, in_maps, list(range(NCORES)),
                                   trace=True)
        LAST_EXEC_NS = res.exec_time_ns
        globals()["LAST_RESULTS"] = res
    else:
        res = run_bass_kernel_spmd(nc, in_maps, list(range(NCORES)))

    out = np.empty((B, S, D), np.float32)
    for c in range(NCORES):
        b, g = divmod(c, 4)
        o = res.results[c]["out"]          # [256 cols, 1024 pos]
        out[b, :, 256 * g:256 * (g + 1)] = o.T
    out += bof[None, None, :]
    return out
